# revision 11
# baseline (speedup 1.0000x reference)
"""Trainium2 Bass kernel for the CSI encoder (complex proj + index embeddings + LayerNorm).

Math: for token t=(b,bs,ue,sc),
  x[d] = real*A[d] + imag*B[d] + C_t[d],  C_t = Base + bs*Ws + ue*Wu + sc*Wf
  out  = (x - mu)/sqrt(var+eps) * gamma + beta
Since C_t is affine in (bs, ue, sc), the LN statistics are scalar functions of
(real, imag) and precomputed per-axis means, and the normalized output tile is
rank-8 per token block:
  out[t, d] = s1*Ag + s2*Bg + s4*gamma + 1*beta + r*BaseG + (bs*r)*Wsg
              + (ue*r)*Wug + (sc*r)*Wfg
with s1=r*real, s2=r*imag, s4=-mu*r, r=rsqrt(var+eps).

Each group of 512 tokens (fixed b,bs,ue) becomes a K=96 x [128, 1024] matmul:
4 "pairs" of 128 tokens live side by side in the output tile (partition p
holds tokens 4p..4p+3), the lhsT holds the 8 per-token stats per pair, and the
rhs is a constant block-diagonal matrix of the 8 d-vectors. FP32 exactness at
1 cyc/row comes from float32r with a hi/lo split: x@y = xh@yh + xh@yl + xl@yh
(sections of 32 K-rows each; the PE's own f32r rounding produces xh from x).

Sharding: data-parallel on the bs_antenna axis, 8 bs per core x 8 cores.
"""

import numpy as np

B, BS, UE, SC, D = 4, 64, 4, 512, 256
NCORES = 8
BSL = BS // NCORES            # 8 bs per core
G = B * BSL * UE              # 128 groups of 512 tokens per core
NP = 512                      # pairs per core (G * 4)
EPS = 1e-5
KSEC = 32                     # K-rows per section (4 pairs x 8 stats)
K = 3 * KSEC                  # lhsT contraction dim per group

_cached = {}


def _build_nc(imms: dict):
    import concourse.bass as bass
    import concourse.mybir as mybir
    from concourse import bacc
    import concourse.tile as tile
    import contextlib

    f32 = mybir.dt.float32
    f32r = mybir.dt.float32r
    mult = mybir.AluOpType.mult
    add = mybir.AluOpType.add
    sub = mybir.AluOpType.subtract

    nc = bacc.Bacc("TRN2", target_bir_lowering=False, debug=False,
                   enable_asserts=False)

    ins = {}
    for name in ("rt", "it", "cbar", "mac2", "mbc2", "mcc", "bsv", "uev", "scv"):
        ins[name] = nc.dram_tensor(name, [128, NP], f32, kind="ExternalInput")
    ins["ybd"] = nc.dram_tensor("ybd", [KSEC, 4 * D], f32, kind="ExternalInput")
    ins["ident"] = nc.dram_tensor("ident", [128, 128], f32, kind="ExternalInput")
    out_t = nc.dram_tensor("out", [G, 128, 4 * D], f32, kind="ExternalOutput")

    with tile.TileContext(nc) as tc:
        with contextlib.ExitStack() as ctx:
            cpool = ctx.enter_context(tc.tile_pool(name="consts", bufs=1))
            tpool = ctx.enter_context(tc.tile_pool(name="tmps", bufs=1))
            ptpool = ctx.enter_context(tc.tile_pool(name="ptp", bufs=1))
            ltpool = ctx.enter_context(tc.tile_pool(name="ltp", bufs=4))
            stpool = ctx.enter_context(tc.tile_pool(name="stage", bufs=4))
            ppool_t = ctx.enter_context(
                tc.tile_pool(name="ps_t", bufs=2, space="PSUM"))
            ppool_o = ctx.enter_context(
                tc.tile_pool(name="ps_o", bufs=3, space="PSUM"))

            ct = {}
            for name in ("rt", "it", "cbar", "mac2", "mbc2", "mcc",
                         "bsv", "uev", "scv"):
                t = cpool.tile([128, NP], f32, name=name)
                nc.sync.dma_start(t[:], ins[name][:, :])
                ct[name] = t
            ybd = cpool.tile([KSEC, 4 * D], f32, name="ybd")
            nc.sync.dma_start(ybd[:], ins["ybd"][:, :])
            ident = cpool.tile([128, 128], f32, name="ident")
            nc.sync.dma_start(ident[:], ins["ident"][:, :])

            # device-side rhs assembly: [yh; yl; yh] sections, f32r-rounded
            yh = cpool.tile([KSEC, 4 * D], f32r, name="yh")
            nc.vector.tensor_copy(yh[:], ybd[:])
            rhs = cpool.tile([K, 4 * D], f32r, name="rhs")
            nc.vector.tensor_copy(rhs[0:KSEC, :], ybd[:])
            nc.vector.tensor_tensor(rhs[KSEC:2 * KSEC, :], ybd[:],
                                    yh[:].bitcast(f32), sub)
            nc.vector.tensor_copy(rhs[2 * KSEC:K, :], ybd[:])

            # PT: pre-transpose stats, col = g*96 + sec*32 + j*8 + s
            pt = ptpool.tile([128, G * K], f32, name="pt")
            ptv = pt[:].rearrange("p (g sec j s) -> p g sec j s",
                                  sec=3, j=4, s=8)

            def slot(sec, s):
                return ptv[:, :, sec, :, s]     # [128, G, 4]

            def v3(t):
                return t[:].rearrange("p (g j) -> p g j", j=4)

            # ---- stats phase (free dim = pair index P over all tokens) ----
            tt = nc.vector.tensor_tensor
            stt = nc.vector.scalar_tensor_tensor
            rt, it = ct["rt"][:], ct["it"][:]

            mu = tpool.tile([128, NP], f32, name="mu")
            ex2 = tpool.tile([128, NP], f32, name="ex2")
            sq = tpool.tile([128, NP], f32, name="sq")
            t1 = tpool.tile([128, NP], f32, name="t1")
            var = tpool.tile([128, NP], f32, name="var")
            sd = tpool.tile([128, NP], f32, name="sd")
            rr = tpool.tile([128, NP], f32, name="rr")
            xhr = tpool.tile([128, NP], f32r, name="xhr")

            stt(mu[:], rt, imms["mA"], ct["cbar"][:], mult, add)
            stt(mu[:], it, imms["mB"], mu[:], mult, add)
            tt(sq[:], rt, it, mult)
            stt(ex2[:], sq[:], imms["mAB2"], ct["mcc"][:], mult, add)
            tt(sq[:], rt, rt, mult)
            stt(ex2[:], sq[:], imms["mAA"], ex2[:], mult, add)
            tt(sq[:], it, it, mult)
            stt(ex2[:], sq[:], imms["mBB"], ex2[:], mult, add)
            tt(t1[:], rt, ct["mac2"][:], mult)
            tt(ex2[:], ex2[:], t1[:], add)
            tt(t1[:], it, ct["mbc2"][:], mult)
            tt(ex2[:], ex2[:], t1[:], add)
            tt(t1[:], mu[:], mu[:], mult)
            tt(var[:], ex2[:], t1[:], sub)
            eps_t = tpool.tile([128, 1], f32, name="eps_t")
            nc.vector.memset(eps_t[:], EPS)
            nc.scalar.activation(sd[:], var[:],
                                 mybir.ActivationFunctionType.Sqrt,
                                 bias=eps_t[:])
            nc.vector.reciprocal(rr[:], sd[:])

            xhr3 = v3(xhr)

            def put(s, compute):
                """compute writes stat value into slot(0,s); then mirror to
                slot(1,s) and write the f32r residual into slot(2,s)."""
                compute(slot(0, s))
                nc.scalar.copy(slot(1, s), slot(0, s))
                nc.vector.tensor_copy(xhr3, slot(0, s))
                tt(slot(2, s), slot(0, s), xhr3.bitcast(f32), sub)

            rr3, rt3, it3, mu3 = v3(rr), v3(ct["rt"]), v3(ct["it"]), v3(mu)
            put(0, lambda o: tt(o, rt3, rr3, mult))                 # s1
            put(1, lambda o: tt(o, it3, rr3, mult))                 # s2
            put(2, lambda o: stt(o, mu3, -1.0, rr3, mult, mult))    # s4
            nc.vector.memset(slot(0, 3), 1.0)                       # ones
            nc.vector.memset(slot(1, 3), 1.0)
            nc.vector.memset(slot(2, 3), 0.0)
            put(4, lambda o: nc.scalar.copy(o, rr3))                # r
            put(5, lambda o: tt(o, v3(ct["bsv"]), rr3, mult))       # bs*r
            put(6, lambda o: tt(o, v3(ct["uev"]), rr3, mult))       # ue*r
            put(7, lambda o: tt(o, v3(ct["scv"]), rr3, mult))       # sc*r

            # ---- per-group: transpose -> 2 K=96 f32r matmuls -> copy -> DMA
            for g in range(G):
                pst = ppool_t.tile([K, 128], f32, tag="pst", name="pst")
                nc.tensor.transpose(pst[:], pt[:, g * K:(g + 1) * K], ident[:])
                ltt = ltpool.tile([K, 128], f32r, tag="ltt", name="ltt")
                nc.vector.tensor_copy(ltt[:], pst[:])

                pso = ppool_o.tile([128, 4 * D], f32, tag="pso", name="pso")
                nc.tensor.matmul(pso[:, :2 * D], ltt[:], rhs[:, :2 * D],
                                 start=True, stop=True)
                nc.tensor.matmul(pso[:, 2 * D:], ltt[:], rhs[:, 2 * D:],
                                 start=True, stop=True)

                stg = stpool.tile([128, 4 * D], f32, tag="stg", name="stg")
                nc.vector.tensor_copy(stg[:, :2 * D], pso[:, :2 * D])
                nc.scalar.copy(stg[:, 2 * D:], pso[:, 2 * D:])
                nc.sync.dma_start(out_t[g], stg[:])
    nc.compile()
    return nc


def _get_nc(imms):
    key = ("nc", tuple(sorted(imms.items())))
    if key not in _cached:
        _cached[key] = _build_nc(imms)
    return _cached[key]


def _host_precompute(inputs):
    f8 = np.float64
    A = inputs["w_c"][:, 0].astype(f8)
    Bv = inputs["w_c"][:, 1].astype(f8)
    b_c = inputs["b_c"].astype(f8)
    w_s = inputs["w_s"][:, 0].astype(f8)
    b_s = inputs["b_s"].astype(f8)
    w_u = inputs["w_u"][:, 0].astype(f8)
    b_u = inputs["b_u"].astype(f8)
    w_f = inputs["w_f"][:, 0].astype(f8)
    b_f = inputs["b_f"].astype(f8)
    g = inputs["ln_gamma"].astype(f8)
    beta = inputs["ln_beta"].astype(f8)

    Base = b_c + b_s + b_u + b_f

    imms = {
        "mA": float(np.mean(A)),
        "mB": float(np.mean(Bv)),
        "mAA": float(np.mean(A * A)),
        "mBB": float(np.mean(Bv * Bv)),
        "mAB2": float(2 * np.mean(A * Bv)),
    }

    # rhs rows: [Ag, Bg, gamma, beta, BaseG, Wsg, Wug, Wfg]; block-diagonal
    # over 4 pairs: ybd[j*8+s, j*256:(j+1)*256] = rows[s]
    rows = np.stack([A * g, Bv * g, g, beta, Base * g, w_s * g, w_u * g,
                     w_f * g]).astype(np.float32)       # [8, D]
    ybd = np.zeros((KSEC, 4 * D), np.float32)
    for j in range(4):
        ybd[j * 8:j * 8 + 8, j * D:(j + 1) * D] = rows

    ident = np.eye(128, dtype=np.float32)

    bs_idx = np.arange(BS, dtype=f8)
    ue_idx = np.arange(UE, dtype=f8)
    sc_idx = np.arange(SC, dtype=f8)

    def tok_mean(vec):
        c0 = np.mean(vec * Base)
        return (c0 + bs_idx[:, None, None] * np.mean(vec * w_s)
                + ue_idx[None, :, None] * np.mean(vec * w_u)
                + sc_idx[None, None, :] * np.mean(vec * w_f))   # [BS,UE,SC]

    cbar = tok_mean(np.ones(D))
    mac2 = 2.0 * tok_mean(A)
    mbc2 = 2.0 * tok_mean(Bv)
    vs = {"b": Base, "s": w_s, "u": w_u, "f": w_f}
    coef = {
        "b": np.ones((BS, UE, SC)),
        "s": np.broadcast_to(bs_idx[:, None, None], (BS, UE, SC)),
        "u": np.broadcast_to(ue_idx[None, :, None], (BS, UE, SC)),
        "f": np.broadcast_to(sc_idx[None, None, :], (BS, UE, SC)),
    }
    mcc = np.zeros((BS, UE, SC))
    for k1 in vs:
        for k2 in vs:
            mcc += coef[k1] * coef[k2] * np.mean(vs[k1] * vs[k2])

    bsv = np.broadcast_to(bs_idx[:, None, None], (BS, UE, SC))
    uev = np.broadcast_to(ue_idx[None, :, None], (BS, UE, SC))
    scv = np.broadcast_to(sc_idx[None, None, :], (BS, UE, SC))

    return imms, ybd, ident, cbar, mac2, mbc2, mcc, bsv, uev, scv


def _to_mP(arr):
    """[B, BSL, UE, SC] -> [128 m, 512 P] with P=g*4+j, sc=4m+j."""
    a = np.ascontiguousarray(arr).reshape(G, SC)
    a = a.reshape(G, 128, 4).transpose(1, 0, 2).reshape(128, NP)
    return np.ascontiguousarray(a, dtype=np.float32)


def _in_maps(inputs):
    (imms, ybd, ident, cbar, mac2, mbc2, mcc,
     bsv, uev, scv) = _host_precompute(inputs)

    cr = np.asarray(inputs["csi_real"], np.float32)
    ci = np.asarray(inputs["csi_imag"], np.float32)

    in_maps = []
    for c in range(NCORES):
        sl = slice(c * BSL, (c + 1) * BSL)

        def percore(tokarr):
            a = np.broadcast_to(tokarr[None, sl], (B, BSL, UE, SC))
            return _to_mP(a)

        m = {
            "rt": _to_mP(cr[:, sl]),
            "it": _to_mP(ci[:, sl]),
            "cbar": percore(cbar),
            "mac2": percore(mac2),
            "mbc2": percore(mbc2),
            "mcc": percore(mcc),
            "bsv": percore(bsv),
            "uev": percore(uev),
            "scv": percore(scv),
            "ybd": ybd,
            "ident": ident,
        }
        in_maps.append(m)
    return imms, in_maps


def _run(inputs, trace=False):
    inputs = {k: np.asarray(v) for k, v in inputs.items()}
    imms, in_maps = _in_maps(inputs)
    nc = _get_nc(imms)

    from concourse.bass_utils import run_bass_kernel_spmd
    res = run_bass_kernel_spmd(nc, in_maps, core_ids=list(range(NCORES)),
                               trace=trace)

    parts = []
    for c in range(NCORES):
        o = res.results[c]["out"].reshape(B, BSL, UE, SC, D)
        parts.append(o)
    full = np.concatenate(parts, axis=1)
    return full, res


def kernel(**inputs):
    full, _ = _run(inputs, trace=False)
    return full


# revision 15
# speedup vs baseline: 331.0038x; 331.0038x over previous
"""Trainium2 Bass kernel for the CSI encoder (complex proj + index embeddings + LayerNorm).

Math: for token t=(b,bs,ue,sc),
  x[d] = real*A[d] + imag*B[d] + C_t[d],  C_t = Base + bs*Ws + ue*Wu + sc*Wf
  out  = (x - mu)/sqrt(var+eps) * gamma + beta
Since C_t is affine in (bs, ue, sc), the LN statistics are scalar functions of
(real, imag) and precomputed per-axis means, and the normalized output tile is
rank-8 per token block:
  out[t, d] = s1*Ag + s2*Bg + s4*gamma + 1*beta + r*BaseG + (bs*r)*Wsg
              + (ue*r)*Wug + (sc*r)*Wfg
with s1=r*real, s2=r*imag, s4=-mu*r, r=rsqrt(var+eps).

Each group of 512 tokens (fixed b,bs,ue) becomes a K=96 x [128, 1024] matmul:
4 "pairs" of 128 tokens live side by side in the output tile (partition p
holds tokens 4p..4p+3), the lhsT holds the 8 per-token stats per pair, and the
rhs is a constant block-diagonal matrix of the 8 d-vectors. FP32 exactness at
1 cyc/row comes from float32r with a hi/lo split: x@y = xh@yh + xh@yl + xl@yh
(sections of 32 K-rows each; the PE's own f32r rounding produces xh from x).

Sharding: data-parallel on the bs_antenna axis, 8 bs per core x 8 cores.
"""

import numpy as np

B, BS, UE, SC, D = 4, 64, 4, 512, 256
NCORES = 8
BSL = BS // NCORES            # 8 bs per core
G = B * BSL * UE              # 128 groups of 512 tokens per core
NP = 512                      # pairs per core (G * 4)
EPS = 1e-5
KSEC = 32                     # K-rows per section (4 pairs x 8 stats)
K = 3 * KSEC                  # lhsT contraction dim per group

_cached = {}


def _build_nc(imms: dict, repeat: int = 1):
    import concourse.bass as bass
    import concourse.mybir as mybir
    from concourse import bacc
    import concourse.tile as tile
    import contextlib

    f32 = mybir.dt.float32
    f32r = mybir.dt.float32r
    mult = mybir.AluOpType.mult
    add = mybir.AluOpType.add
    sub = mybir.AluOpType.subtract

    nc = bacc.Bacc("TRN2", target_bir_lowering=False, debug=False,
                   enable_asserts=False)

    ins = {}
    for name in ("rt", "it", "cbar", "mac2", "mbc2", "mcc", "bsv", "uev", "scv"):
        ins[name] = nc.dram_tensor(name, [128, NP], f32, kind="ExternalInput")
    ins["ybd"] = nc.dram_tensor("ybd", [KSEC, 4 * D], f32, kind="ExternalInput")
    ins["ident"] = nc.dram_tensor("ident", [128, 128], f32, kind="ExternalInput")
    out_t = nc.dram_tensor("out", [G, 128, 4 * D], f32, kind="ExternalOutput")

    with tile.TileContext(nc) as tc:
        with contextlib.ExitStack() as ctx:
            cpool = ctx.enter_context(tc.tile_pool(name="consts", bufs=1))
            tpool = ctx.enter_context(tc.tile_pool(name="tmps", bufs=1))
            ptpool = ctx.enter_context(tc.tile_pool(name="ptp", bufs=1))
            ltpool = ctx.enter_context(tc.tile_pool(name="ltp", bufs=4))
            stpool = ctx.enter_context(tc.tile_pool(name="stage", bufs=4))
            ppool_t = ctx.enter_context(
                tc.tile_pool(name="ps_t", bufs=2, space="PSUM"))
            ppool_o = ctx.enter_context(
                tc.tile_pool(name="ps_o", bufs=3, space="PSUM"))

            ct = {}
            for name in ("rt", "it", "cbar", "mac2", "mbc2", "mcc",
                         "bsv", "uev", "scv"):
                t = cpool.tile([128, NP], f32, name=name)
                nc.sync.dma_start(t[:], ins[name][:, :])
                ct[name] = t
            ybd = cpool.tile([KSEC, 4 * D], f32, name="ybd")
            nc.sync.dma_start(ybd[:], ins["ybd"][:, :])
            ident = cpool.tile([128, 128], f32, name="ident")
            nc.sync.dma_start(ident[:], ins["ident"][:, :])

            # device-side rhs assembly: [yh; yl; yh] sections, f32r-rounded
            yh = cpool.tile([KSEC, 4 * D], f32r, name="yh")
            nc.vector.tensor_copy(yh[:], ybd[:])
            rhs = cpool.tile([K, 4 * D], f32r, name="rhs")
            nc.vector.tensor_copy(rhs[0:KSEC, :], ybd[:])
            nc.vector.tensor_tensor(rhs[KSEC:2 * KSEC, :], ybd[:],
                                    yh[:].bitcast(f32), sub)
            nc.vector.tensor_copy(rhs[2 * KSEC:K, :], ybd[:])

            # PT: pre-transpose stats, col = g*96 + sec*32 + j*8 + s
            pt = ptpool.tile([128, G * K], f32, name="pt")
            ptv = pt[:].rearrange("p (g sec j s) -> p g sec j s",
                                  sec=3, j=4, s=8)

            def slot(sec, s):
                return ptv[:, :, sec, :, s]     # [128, G, 4]

            def v3(t):
                return t[:].rearrange("p (g j) -> p g j", j=4)

            tt = nc.vector.tensor_tensor
            stt = nc.vector.scalar_tensor_tensor
            rt, it = ct["rt"][:], ct["it"][:]

            mu = tpool.tile([128, NP], f32, name="mu")
            ex2 = tpool.tile([128, NP], f32, name="ex2")
            sq = tpool.tile([128, NP], f32, name="sq")
            t1 = tpool.tile([128, NP], f32, name="t1")
            var = tpool.tile([128, NP], f32, name="var")
            sd = tpool.tile([128, NP], f32, name="sd")
            rr = tpool.tile([128, NP], f32, name="rr")
            xhr = tpool.tile([128, NP], f32r, name="xhr")
            eps_t = tpool.tile([128, 1], f32, name="eps_t")
            nc.vector.memset(eps_t[:], EPS)

            loop_cm = (tc.For_i(0, repeat, 1) if repeat > 1
                       else contextlib.nullcontext())
            ctx.enter_context(loop_cm)

            # ---- stats phase (free dim = pair index P over all tokens) ----
            stt(mu[:], rt, imms["mA"], ct["cbar"][:], mult, add)
            stt(mu[:], it, imms["mB"], mu[:], mult, add)
            tt(sq[:], rt, it, mult)
            stt(ex2[:], sq[:], imms["mAB2"], ct["mcc"][:], mult, add)
            tt(sq[:], rt, rt, mult)
            stt(ex2[:], sq[:], imms["mAA"], ex2[:], mult, add)
            tt(sq[:], it, it, mult)
            stt(ex2[:], sq[:], imms["mBB"], ex2[:], mult, add)
            tt(t1[:], rt, ct["mac2"][:], mult)
            tt(ex2[:], ex2[:], t1[:], add)
            tt(t1[:], it, ct["mbc2"][:], mult)
            tt(ex2[:], ex2[:], t1[:], add)
            tt(t1[:], mu[:], mu[:], mult)
            tt(var[:], ex2[:], t1[:], sub)
            nc.scalar.activation(sd[:], var[:],
                                 mybir.ActivationFunctionType.Sqrt,
                                 bias=eps_t[:])
            nc.vector.reciprocal(rr[:], sd[:])

            xhr3 = v3(xhr)

            def put(s, compute):
                """compute writes stat value into slot(0,s); then mirror to
                slot(1,s) and write the f32r residual into slot(2,s)."""
                compute(slot(0, s))
                nc.scalar.copy(slot(1, s), slot(0, s))
                nc.vector.tensor_copy(xhr3, slot(0, s))
                tt(slot(2, s), slot(0, s), xhr3.bitcast(f32), sub)

            rr3, rt3, it3, mu3 = v3(rr), v3(ct["rt"]), v3(ct["it"]), v3(mu)
            put(0, lambda o: tt(o, rt3, rr3, mult))                 # s1
            put(1, lambda o: tt(o, it3, rr3, mult))                 # s2
            put(2, lambda o: stt(o, mu3, -1.0, rr3, mult, mult))    # s4
            nc.vector.memset(slot(0, 3), 1.0)                       # ones
            nc.vector.memset(slot(1, 3), 1.0)
            nc.vector.memset(slot(2, 3), 0.0)
            put(4, lambda o: nc.scalar.copy(o, rr3))                # r
            put(5, lambda o: tt(o, v3(ct["bsv"]), rr3, mult))       # bs*r
            put(6, lambda o: tt(o, v3(ct["uev"]), rr3, mult))       # ue*r
            put(7, lambda o: tt(o, v3(ct["scv"]), rr3, mult))       # sc*r

            # ---- per-group: transpose -> 2 K=96 f32r matmuls -> copy -> DMA
            for g in range(G):
                pst = ppool_t.tile([K, 128], f32, tag="pst", name="pst")
                nc.tensor.transpose(pst[:], pt[:, g * K:(g + 1) * K], ident[:])
                ltt = ltpool.tile([K, 128], f32r, tag="ltt", name="ltt")
                nc.vector.tensor_copy(ltt[:], pst[:])

                pso = ppool_o.tile([128, 4 * D], f32, tag="pso", name="pso")
                nc.tensor.matmul(pso[:, :2 * D], ltt[:], rhs[:, :2 * D],
                                 start=True, stop=True)
                nc.tensor.matmul(pso[:, 2 * D:], ltt[:], rhs[:, 2 * D:],
                                 start=True, stop=True)

                stg = stpool.tile([128, 4 * D], f32, tag="stg", name="stg")
                nc.vector.tensor_copy(stg[:, :2 * D], pso[:, :2 * D])
                nc.scalar.copy(stg[:, 2 * D:], pso[:, 2 * D:])
                nc.sync.dma_start(out_t[g], stg[:])
    nc.compile()
    return nc


def _get_nc(imms, repeat=1):
    key = ("nc", repeat, tuple(sorted(imms.items())))
    if key not in _cached:
        _cached[key] = _build_nc(imms, repeat)
    return _cached[key]


def _host_precompute(inputs):
    f8 = np.float64
    A = inputs["w_c"][:, 0].astype(f8)
    Bv = inputs["w_c"][:, 1].astype(f8)
    b_c = inputs["b_c"].astype(f8)
    w_s = inputs["w_s"][:, 0].astype(f8)
    b_s = inputs["b_s"].astype(f8)
    w_u = inputs["w_u"][:, 0].astype(f8)
    b_u = inputs["b_u"].astype(f8)
    w_f = inputs["w_f"][:, 0].astype(f8)
    b_f = inputs["b_f"].astype(f8)
    g = inputs["ln_gamma"].astype(f8)
    beta = inputs["ln_beta"].astype(f8)

    Base = b_c + b_s + b_u + b_f

    imms = {
        "mA": float(np.mean(A)),
        "mB": float(np.mean(Bv)),
        "mAA": float(np.mean(A * A)),
        "mBB": float(np.mean(Bv * Bv)),
        "mAB2": float(2 * np.mean(A * Bv)),
    }

    # rhs rows: [Ag, Bg, gamma, beta, BaseG, Wsg, Wug, Wfg]; block-diagonal
    # over 4 pairs: ybd[j*8+s, j*256:(j+1)*256] = rows[s]
    rows = np.stack([A * g, Bv * g, g, beta, Base * g, w_s * g, w_u * g,
                     w_f * g]).astype(np.float32)       # [8, D]
    ybd = np.zeros((KSEC, 4 * D), np.float32)
    for j in range(4):
        ybd[j * 8:j * 8 + 8, j * D:(j + 1) * D] = rows

    ident = np.eye(128, dtype=np.float32)

    bs_idx = np.arange(BS, dtype=f8)
    ue_idx = np.arange(UE, dtype=f8)
    sc_idx = np.arange(SC, dtype=f8)

    def tok_mean(vec):
        c0 = np.mean(vec * Base)
        return (c0 + bs_idx[:, None, None] * np.mean(vec * w_s)
                + ue_idx[None, :, None] * np.mean(vec * w_u)
                + sc_idx[None, None, :] * np.mean(vec * w_f))   # [BS,UE,SC]

    cbar = tok_mean(np.ones(D))
    mac2 = 2.0 * tok_mean(A)
    mbc2 = 2.0 * tok_mean(Bv)
    vs = {"b": Base, "s": w_s, "u": w_u, "f": w_f}
    coef = {
        "b": np.ones((BS, UE, SC)),
        "s": np.broadcast_to(bs_idx[:, None, None], (BS, UE, SC)),
        "u": np.broadcast_to(ue_idx[None, :, None], (BS, UE, SC)),
        "f": np.broadcast_to(sc_idx[None, None, :], (BS, UE, SC)),
    }
    mcc = np.zeros((BS, UE, SC))
    for k1 in vs:
        for k2 in vs:
            mcc += coef[k1] * coef[k2] * np.mean(vs[k1] * vs[k2])

    bsv = np.broadcast_to(bs_idx[:, None, None], (BS, UE, SC))
    uev = np.broadcast_to(ue_idx[None, :, None], (BS, UE, SC))
    scv = np.broadcast_to(sc_idx[None, None, :], (BS, UE, SC))

    return imms, ybd, ident, cbar, mac2, mbc2, mcc, bsv, uev, scv


def _to_mP(arr):
    """[B, BSL, UE, SC] -> [128 m, 512 P] with P=g*4+j, sc=4m+j."""
    a = np.ascontiguousarray(arr).reshape(G, SC)
    a = a.reshape(G, 128, 4).transpose(1, 0, 2).reshape(128, NP)
    return np.ascontiguousarray(a, dtype=np.float32)


def _in_maps(inputs):
    (imms, ybd, ident, cbar, mac2, mbc2, mcc,
     bsv, uev, scv) = _host_precompute(inputs)

    cr = np.asarray(inputs["csi_real"], np.float32)
    ci = np.asarray(inputs["csi_imag"], np.float32)

    in_maps = []
    for c in range(NCORES):
        sl = slice(c * BSL, (c + 1) * BSL)

        def percore(tokarr):
            a = np.broadcast_to(tokarr[None, sl], (B, BSL, UE, SC))
            return _to_mP(a)

        m = {
            "rt": _to_mP(cr[:, sl]),
            "it": _to_mP(ci[:, sl]),
            "cbar": percore(cbar),
            "mac2": percore(mac2),
            "mbc2": percore(mbc2),
            "mcc": percore(mcc),
            "bsv": percore(bsv),
            "uev": percore(uev),
            "scv": percore(scv),
            "ybd": ybd,
            "ident": ident,
        }
        in_maps.append(m)
    return imms, in_maps


def _run(inputs, trace=False):
    inputs = {k: np.asarray(v) for k, v in inputs.items()}
    imms, in_maps = _in_maps(inputs)
    nc = _get_nc(imms)

    from concourse.bass_utils import run_bass_kernel_spmd
    res = run_bass_kernel_spmd(nc, in_maps, core_ids=list(range(NCORES)),
                               trace=trace)

    parts = []
    for c in range(NCORES):
        o = res.results[c]["out"].reshape(B, BSL, UE, SC, D)
        parts.append(o)
    full = np.concatenate(parts, axis=1)
    return full, res


def kernel(**inputs):
    full, _ = _run(inputs, trace=False)
    return full
